# revision 3
# baseline (speedup 1.0000x reference)
"""Trainium2 Bass kernel for nn_DecodingLoss (cepstrum decoding loss), v2.

Math (per 4096-sample window):
  cep = irfft(log(|rfft(x)| + eps))[DELAYS]; softargmax(beta=1e10) -> argmax idx;
  loss = clip(|idx - symbol|,0,1); per-audio sums -> 5 scalar outputs.

v2 changes over the baseline:
  - Real-input conjugate symmetry in stage 1: At[s,u] for u=0..16 only
    (At[s,32-u] = conj(At[s,u])), halving stage-1 PE matmul columns
    (136 vs 256 per 4-window group) and halving the PSUM->SBUF `at`
    evacuation volume.  Stage-2 "units" cover all 2048 bins via
    k = c + 32(v + 64*half) per class c (cos/e^{i2pi k s/4096} is 4096-
    periodic in k, so v>=64 rows reuse the same machinery; |X[4096-k]| =
    |X[k]| covers residues 17..31).
  - Fold (|X|^2 partition fold) as ONE 512-column matmul per p instead of
    two 256-column ones (half the PE fold instructions/ldweights).
  - Squares on ACT (PSUM-drain); at-evac on DVE (prologue alternates
    DVE/ACT while ACT is otherwise idle).
  - Batched tail: cep accumulates in PSUM [8, 512] across 2 iterations,
    drained twice; ONE argmax pass (reduce_max + max_index over
    [128, 8 chunks x 8 taps]) replaces the per-128-window softargmax op
    chain.  beta=1e10 makes softargmax == argmax (ties are measure-zero).
    Host precomputes symoff[p,a] = 8a + symbol so loss = min(|idx-symoff|,1).
"""
import numpy as np
import ml_dtypes

import concourse.bass as bass
import concourse.mybir as mybir
from concourse import tile
from concourse.bass_utils import run_bass_kernel_spmd

FP32 = mybir.dt.float32
BF16 = mybir.dt.bfloat16
I32 = mybir.dt.int32
U32 = mybir.dt.uint32
F8 = mybir.dt.float8e4

B, NW, WIN = 64, 128, 4096
NCORES = 8
BLOC = B // NCORES              # 8 audio rows per core
WLOC = BLOC * NW                # 1024 windows per core
T, S, U = 32, 128, 17           # n = 128 t + s ; u-classes 0..16 (conj sym)
NV = 64
ITERS = 4
WPI = WLOC // ITERS             # 256 windows per iteration
G = WPI // 4                    # 64 groups of 4 windows
CGRP = 2 * U * 4                # 136 stage-1 moving cols per 4-window group
DELAYS = np.array([64, 96, 128, 160, 192, 224, 256, 288])

_cache = {}


def _hoist_waits(bir_json):
    """This walrus build rejects instructions carrying attached semaphore waits
    ("Too many sync wait commands"); raw-bass style standalone EventSemaphore
    waits compile and run. Hoist every attached wait into its own
    EventSemaphore on the same engine queue; updates stay attached.
    HOISTK=n keeps up to n waits attached (experiment: attached waits may
    let the PE reorder window pull LDWEIGHTS ahead)."""
    import json, os
    keep = int(os.environ.get("HOISTK", "1"))
    d = json.loads(bir_json)
    n = 0
    for fn in d["functions"]:
        for bb in fn["blocks"]:
            out = []
            for ins in bb["instructions"]:
                si = ins.get("sync_info")
                waits = (si or {}).get("on_wait") or []
                if (len(waits) > keep and ins.get("opcode") != "EventSemaphore"
                        and ins.get("engine")):
                    for w in waits[:-keep] if keep else waits:
                        n += 1
                        out.append({
                            "name": f"hoistw-{n}", "opcode": "EventSemaphore",
                            "engine": ins["engine"], "ins": [], "outs": [],
                            "sync_info": {"on_wait": [w], "on_update": []},
                        })
                    si["on_wait"] = waits[-keep:] if keep else []
                out.append(ins)
            bb["instructions"] = out
    return json.dumps(d).encode()


def _install_hoist(nc):
    orig = nc.to_json_bytes
    nc.to_json_bytes = lambda: _hoist_waits(orig())
    return nc
LINEARIZE = False


def _slot_specs():
    """16 stage-2 slots. Slot 0 packs classes 0 (rows 0:64) and 16
    (rows 64:128); slots 1..15 are class c with v=0..127 (k = c+32v;
    k>2048 rows alias bins 4096-k). Each slot's two DR matmuls emit pure
    Xre (psX cols 0:256) and pure Xim (cols 256:512) across v-partitions,
    so |X|^2 is a same-partition column add -- no PE fold needed."""
    specs = []
    for sl in range(16):
        if sl == 0:
            specs.append([(0, 32 * (np.arange(64) + 1), 0),
                          (16, 16 + 32 * np.arange(64), 64)])
        else:
            specs.append([(sl, sl + 32 * np.arange(128), 0)])
    return specs


def _tables():
    t = np.arange(T)[:, None]
    u = np.arange(U)[None, :]
    c32 = np.cos(2 * np.pi * t * u / 32.0)          # [t, u17]
    s32n = -np.sin(2 * np.pi * t * u / 32.0)

    # stage-1 moving operand: rows (w4, t); cols (c, u17, w4') block-diag
    bdcs = np.zeros((4, T, 2, U, 4), np.float64)
    for w4 in range(4):
        bdcs[w4, :, 0, :, w4] = c32
        bdcs[w4, :, 1, :, w4] = s32n
    bdcs = bdcs.reshape(128, CGRP)

    s = np.arange(S)[:, None]
    # stage-2 stationaries: 17 blocks of [S, RI(2), m(2), 128]; m=0 applies
    # to At_re, m=1 to At_im.  RI=0 -> Xre[v] rows, RI=1 -> Xim[v] rows.
    # Block j = slot j for j<16; block 16 = class 16 (slot 0's second
    # accumulated matmul), zero-padded so out rows land at 64:128 with
    # tile_position (0,0) -- DoubleRow rejects a nonzero out base partition.
    h2m = np.zeros((S, 17, 2, 2, 128), np.float64)
    # projection: cep[d] = sum_k wk*0.5*log(m2)[k]*cos(2 pi k d/4096)/4096
    pp = np.zeros((128, 16, 8), np.float64)
    for sl, specs in enumerate(_slot_specs()):
        for cls, kg, fo in specs:
            nv = kg.shape[0]
            jb = 16 if (sl == 0 and cls == 16) else sl
            ph = 2 * np.pi * s * kg[None, :] / 4096.0
            h2m[:, jb, 0, 0, fo:fo + nv] = np.cos(ph)
            h2m[:, jb, 0, 1, fo:fo + nv] = np.sin(ph)
            h2m[:, jb, 1, 0, fo:fo + nv] = -np.sin(ph)
            h2m[:, jb, 1, 1, fo:fo + nv] = np.cos(ph)
            keff = np.where(kg <= 2048, kg, 4096 - kg)
            wk = np.where(keff == 2048, 1.0, 2.0)
            for j, d in enumerate(DELAYS):
                pp[fo:fo + nv, sl, j] = (
                    wk * 0.5 * np.cos(2 * np.pi * kg * d / 4096.0) / 4096.0)
    h2m = h2m.reshape(S, 17 * 2 * 2 * 128)
    ident8 = np.eye(8, dtype=np.float32)
    return (bdcs.astype(ml_dtypes.bfloat16), h2m.astype(ml_dtypes.float8_e4m3),
            pp.astype(ml_dtypes.bfloat16), ident8)


def _build():
    slot_specs = _slot_specs()

    nc = bass.Bass()
    audio = nc.dram_tensor("audio", [WLOC, WIN], BF16, kind="ExternalInput")
    symoff_d = nc.dram_tensor("symoff", [128, BLOC], FP32, kind="ExternalInput")
    bdcs_d = nc.dram_tensor("bdcs", [128, CGRP], BF16, kind="ExternalInput")
    h2m_d = nc.dram_tensor("h2m", [S, 17 * 512], F8, kind="ExternalInput")
    pp_d = nc.dram_tensor("pp", [128, 16, 8], BF16, kind="ExternalInput")
    id8_d = nc.dram_tensor("ident8", [8, 8], FP32, kind="ExternalInput")
    loss_out = nc.dram_tensor("loss_out", [128, BLOC], FP32,
                              kind="ExternalOutput")

    with tile.TileContext(nc, linearize=LINEARIZE) as tc:
        with (
            tc.tile_pool(name="consts", bufs=1) as consts,
            tc.tile_pool(name="xt", bufs=2) as xt_pool,
            tc.tile_pool(name="at", bufs=3) as at_pool,
            tc.tile_pool(name="sq", bufs=5) as sq_pool,
            tc.tile_pool(name="m2", bufs=3) as m2_pool,
            tc.tile_pool(name="lg", bufs=4) as lg_pool,
            tc.tile_pool(name="fin", bufs=2) as fin_pool,
            tc.tile_pool(name="cs", bufs=2) as cs_pool,
            tc.tile_pool(name="ps1", bufs=2, space="PSUM") as ps1_pool,
            tc.tile_pool(name="psX", bufs=3, space="PSUM") as psX_pool,
            tc.tile_pool(name="cep", bufs=1, space="PSUM") as cep_pool,
        ):
            bdcs = consts.tile([128, CGRP], BF16, tag="bdcs")
            h2m = consts.tile([128, 17 * 512], F8, tag="h2m")
            ppj = consts.tile([128, 128], BF16, tag="ppj")
            ident8 = consts.tile([8, 8], FP32, tag="ident8")
            symoff = consts.tile([128, BLOC], FP32, tag="symoff")
            epsb = consts.tile([128, 1], FP32, tag="epsb")
            nc.vector.memset(epsb[:], 1e-10)

            def load_consts_rest():
                nc.sync.dma_start(h2m[:], h2m_d[:])
                nc.sync.dma_start(ppj[:], pp_d[:].rearrange("s p j -> s (p j)"))
                nc.sync.dma_start(ident8[:], id8_d[:])
                nc.sync.dma_start(symoff[:], symoff_d[:])

            xts, ats = {}, {}

            def dma_in(it, nchunks=4):
                # alternate chunks between the SP and ACT hardware DGE
                # queues so audio-in bandwidth is not single-queue bound
                xt = xt_pool.tile([128, WPI * 32], BF16, tag="xt")
                cw = WPI // nchunks
                for j in range(nchunks):
                    nc.sync.dma_start(
                        xt[:, j * cw * 32:(j + 1) * cw * 32]
                        .rearrange("p (g s) -> p g s", s=S),
                        audio[it * WPI + j * cw:it * WPI + (j + 1) * cw, :]
                        .rearrange("(g w4) (t s) -> (w4 t) g s", w4=4, s=S))
                xts[it] = xt

            def s1_start(it):
                # at cols: (u17, c2, g64, w4) -> per-class 512-col slices
                at = at_pool.tile([128, U * 512], F8, tag="at")
                atv = at[:].rearrange("s (u c g w) -> s c u g w", u=U, c=2, w=4)
                ats[it] = at
                return (xts[it], atv)

            def s1_unit(st, b, act_evac=False):
                # one 4-group unit: 4 matmuls ([128,136] out each, 256-col
                # stride in a 2-bank psum tile) + 1 evac op (whole unit)
                xt, atv = st
                ps1 = ps1_pool.tile([128, 1024], FP32, tag="ps1")
                for g4 in range(4):
                    nc.tensor.matmul(ps1[:, g4 * 256:g4 * 256 + CGRP],
                                     xt[:, (4 * b + g4) * 128:
                                        (4 * b + g4 + 1) * 128],
                                     bdcs[:], start=True, stop=True)
                ps1v = (ps1[:].rearrange("s (g x) -> s g x", g=4)[:, :, 0:CGRP]
                        .rearrange("s g (c u w) -> s c u g w", c=2, w=4))
                dst = atv[:, :, :, 4 * b:4 * b + 4, :]
                if act_evac:
                    nc.scalar.activation(dst, ps1v,
                                         mybir.ActivationFunctionType.Copy)
                else:
                    nc.vector.tensor_copy(dst, ps1v)

            def s1_block(it):
                # ACT is otherwise idle here: alternate evac engines
                st = s1_start(it)
                for b in range(G // 4):
                    s1_unit(st, b, act_evac=(b % 2 == 1))

            def s2_block(it, cep, inter=()):
                at = ats.pop(it)
                sqs, m2s, lgs = {}, {}, {}
                coff = (it % 2) * 256

                def s2mm(sl):
                    # psX cols 0:256 = Xre[v-rows], 256:512 = Xim[v-rows];
                    # slot 0 accumulates class 0 (rows 0:64, zero-padded
                    # stat block 0) + class 16 (rows 64:128, block 16)
                    psX = psX_pool.tile([128, 512], FP32, tag="psX")
                    parts = ([(0, 0, True), (16, 16, False)] if sl == 0
                             else [(sl, sl, True)])
                    for jb, cls, first in parts:
                        last = (jb != 0) if sl == 0 else True
                        mov = (at[:, cls * 512:(cls + 1) * 512]
                               .rearrange("s (c w) -> s c w", c=2))
                        for ri in range(2):
                            stat = (h2m[:, (jb * 4 + ri * 2) * 128:
                                        (jb * 4 + ri * 2 + 2) * 128]
                                    .rearrange("s (c f) -> s c f", c=2))
                            nc.tensor.matmul(
                                psX[:, ri * 256:(ri + 1) * 256],
                                stat, mov, start=first, stop=last,
                                perf_mode=mybir.MatmulPerfMode.DoubleRow)
                    sq = sq_pool.tile([128, 512], BF16, tag="sq")
                    nc.scalar.activation(sq[:], psX[:],
                                         mybir.ActivationFunctionType.Square)
                    sqs[sl] = sq

                def m2add(sl):
                    # |X|^2 = Xre^2 + Xim^2: same-partition column add (DVE
                    # 2-byte fast path); 2 slots pack one [128,512] m2 tile
                    if sl % 2 == 0:
                        m2 = m2_pool.tile([128, 512], BF16, tag="m2")
                        m2s[sl // 2] = m2
                    else:
                        m2 = m2s[sl // 2]
                    sq = sqs.pop(sl)
                    nc.vector.tensor_add(m2[:, (sl % 2) * 256:(sl % 2) * 256 + 256],
                                         sq[:, 0:256], sq[:, 256:512])
                    if sl % 2 == 1:
                        lg = lg_pool.tile([128, 512], BF16, tag="lg")
                        nc.scalar.activation(lg[:], m2s.pop(sl // 2)[:],
                                             mybir.ActivationFunctionType.Ln,
                                             bias=epsb[:])
                        lgs[sl // 2] = lg

                def proj(sl):
                    lg = lgs[sl // 2][:, (sl % 2) * 256:(sl % 2) * 256 + 256]
                    if sl % 2 == 1:
                        lgs.pop(sl // 2)
                    nc.tensor.matmul(cep[0:8, coff:coff + 256],
                                     ppj[:, sl * 8:(sl + 1) * 8],
                                     lg, start=(sl == 0), stop=(sl == 15))

                for sl in range(16):
                    s2mm(sl)
                    if sl < len(inter):
                        s1_unit(*inter[sl])
                    if sl >= 2:
                        m2add(sl - 2)
                    if sl >= 6:
                        proj(sl - 6)
                m2add(14)
                m2add(15)
                for sl in range(10, 16):
                    proj(sl)

            # ---------------- schedule ----------------
            # short prologue (iter 0 only); iters 1-3 stage-1 spreads into
            # the s2 p-loops of blocks 0-2 so every engine stays fed
            nc.sync.dma_start(bdcs[:], bdcs_d[:])
            dma_in(0, nchunks=8)
            dma_in(1, nchunks=8)
            load_consts_rest()
            s1_block(0)
            cep = cep_pool.tile([128, 512], FP32, tag="cep")
            st1 = s1_start(1)
            dma_in(2)
            s2_block(0, cep, inter=[(st1, b) for b in range(16)])
            st2 = s1_start(2)
            dma_in(3)
            s2_block(1, cep, inter=[(st2, b) for b in range(16)])
            cs0 = cs_pool.tile([8, 512], FP32, tag="cs")
            nc.vector.tensor_copy(cs0[:], cep[0:8, 0:512])
            st3 = s1_start(3)
            s2_block(2, cep, inter=[(st3, b) for b in range(16)])
            s2_block(3, cep)
            cs1 = cs_pool.tile([8, 512], FP32, tag="cs")
            nc.vector.tensor_copy(cs1[:], cep[0:8, 0:512])

            # ---------------- batched tail ----------------
            # chunk c (=audio index) = windows 128c..128c+127;
            # chunks 0..3 from cs0 (iters 0,1), 4..7 from cs1.
            # reuse the (now dead) cep bank for the transposed taps
            psT = cep_pool.tile([128, 512], FP32, tag="cep")
            for c in range(8):
                src = cs0 if c < 4 else cs1
                nc.tensor.transpose(psT[:, c * 8:(c + 1) * 8],
                                    src[:, (c % 4) * 128:(c % 4 + 1) * 128],
                                    ident8[:])
            csb = fin_pool.tile([128, 64], FP32, tag="csb")
            nc.vector.tensor_copy(csb[:], psT[:, 0:64])
            mx = fin_pool.tile([128, 8], FP32, tag="mx")
            nc.vector.reduce_max(mx[:], csb[:].rearrange("p (c t) -> p c t", t=8),
                                 axis=mybir.AxisListType.X)
            mi = fin_pool.tile([128, 8], U32, tag="mi")
            nc.vector.max_index(mi[:], mx[:], csb[:])
            mif = fin_pool.tile([128, 8], FP32, tag="mif")
            nc.vector.tensor_copy(mif[:], mi[:])
            df = fin_pool.tile([128, 8], FP32, tag="df")
            nc.vector.tensor_sub(df[:], mif[:], symoff[:])
            ab = fin_pool.tile([128, 8], FP32, tag="ab")
            nc.scalar.activation(ab[:], df[:],
                                 mybir.ActivationFunctionType.Abs)
            ls = fin_pool.tile([128, 8], FP32, tag="ls")
            nc.vector.tensor_scalar_min(ls[:], ab[:], 1.0)
            nc.sync.dma_start(loss_out[:], ls[:])
    return nc


def kernel(audio_batch, symbols_batch, num_errs_no_reverb_batch,
           num_errs_reverb_batch):
    audio_batch = np.asarray(audio_batch)
    symbols_batch = np.asarray(symbols_batch, dtype=np.int32)
    nn_ = np.asarray(num_errs_no_reverb_batch).astype(np.float32)
    nr_ = np.asarray(num_errs_reverb_batch).astype(np.float32)

    if "nc" not in _cache:
        _cache["nc"] = _install_hoist(_build())
        _cache["tabs"] = _tables()
    nc = _cache["nc"]
    bdcs, h2m, pp, ident8 = _cache["tabs"]

    audio_bf = (audio_batch.reshape(B, NW * WIN)
                .astype(ml_dtypes.bfloat16)
                .reshape(NCORES, WLOC, WIN))
    syms = symbols_batch.reshape(NCORES, BLOC, NW)
    in_maps = []
    for c in range(NCORES):
        # symoff[p, a] = 8*a + symbol(window a*128+p)
        so = (8.0 * np.arange(BLOC)[None, :]
              + syms[c].T.astype(np.float32)).astype(np.float32)
        in_maps.append({
            "audio": audio_bf[c], "symoff": so,
            "bdcs": bdcs, "h2m": h2m, "pp": pp, "ident8": ident8,
        })
    import os
    res = run_bass_kernel_spmd(nc, in_maps, core_ids=list(range(NCORES)),
                               trace=bool(os.environ.get("KTRACE")))
    _cache["last_res"] = res
    errs_all = []
    for c in range(NCORES):
        loss = res.results[c]["loss_out"]          # [128, 8]
        errs_all.append(loss.sum(axis=0, dtype=np.float32))
    errs = np.concatenate(errs_all)                 # [64]

    tot = np.float32(errs.sum())
    diff = nr_ - nn_
    inv_red = np.where(diff == 0, np.float32(1.0), diff / (nr_ - errs))
    ter = np.float32(inv_red.sum())
    denom = np.float32(B * NW)
    return (np.float32(tot / denom), tot, np.float32(ter / B),
            np.float32(nn_.sum() / denom), np.float32(nr_.sum() / denom))
